# revision 6
# baseline (speedup 1.0000x reference)
"""Trainium2 Bass kernel for nn_BodyKinematics (batched tree forward kinematics).

Contract: kernel(**inputs) takes the FULL unsharded inputs as numpy arrays and
returns the FULL output (B, N, 4, 4) float32.  Internally the batch dim is
sharded across 8 NeuronCores (pure data parallelism); the tiny per-edge
parameters are replicated.

Math (matches the jax reference):
  theta = tanh(log_angles) * scale + offset            # (B, 3E)
  per edge e: r = Rx(th_x) @ Ry(th_y) @ Rz(th_z)       # axes are e_x, e_y, e_z
  local_e  = [r | 0; 0 1] @ tip_to_base[e]             # affine 3x4 is enough
  tree: W_0 = I, W_n = W_parent(n) @ local_{n-1}       # parent(n) = (n-1)//2
  output: W as 4x4 with constant bottom row (0,0,0,1)

Device layout (per core, 512 batch rows = 4 subtiles x 128 partitions):
  partitions = batch-within-subtile, free dim = per-edge structure.
  locals tile (per sub):  [128, E*12]   (e, i, l) row-major
  worlds tile (per sub):  [128, N*16]   (n, i, l) -> 16KB contiguous per batch
                                         row for efficient output DMA
"""

import os
import sys

for _p in ("/opt/trn_rl_repo",):
    if _p not in sys.path and os.path.isdir(_p):
        sys.path.insert(0, _p)

import numpy as np

B, E, N = 4096, 255, 256
J = 3 * E           # 765 angles
NCORE, P, NSUB = 8, 128, 4
BPC = P * NSUB      # 512 batch rows per core
PI = float(np.pi)

# engine assignment per subtile for the two heavy stages: "v" = VectorE (DVE),
# "g" = GpSimd (POOL).  Tuned empirically.
ENG_BC = ["v", "v", "v", "v"]
ENG_TREE = ["v", "v", "v", "v"]

_state: dict = {}


# --------------------------------------------------------------------------- #
# numpy fallback (exact float32 port of the reference) — used only if the
# inputs don't match the structure the device kernel was built for.
# --------------------------------------------------------------------------- #
def _np_skew(a):
    x, y, z = a[..., 0], a[..., 1], a[..., 2]
    zero = np.zeros_like(x)
    return np.stack([
        np.stack([zero, -z, y], -1),
        np.stack([z, zero, -x], -1),
        np.stack([-y, x, zero], -1)], -2)


def _np_fallback(log_angles, tip_to_base, rot_axes, rot_constraints):
    la = log_angles.astype(np.float32)
    b, e3 = la.shape
    e = e3 // 3
    n = e + 1
    theta = np.tanh(la) * rot_constraints[:, 0] + rot_constraints[:, 1]
    K = _np_skew(rot_axes.astype(np.float32))
    K2 = np.einsum('mij,mjk->mik', K, K).astype(np.float32)
    s = np.sin(theta)[..., None, None]
    c = (1.0 - np.cos(theta))[..., None, None]
    I3 = np.eye(3, dtype=np.float32)
    rots = (I3 + s * K + c * K2).reshape(b, e, 3, 3, 3).astype(np.float32)
    r = np.einsum('beij,bejk,bekl->beil', rots[:, :, 0], rots[:, :, 1],
                  rots[:, :, 2]).astype(np.float32)
    T = np.zeros((b, e, 4, 4), np.float32)
    T[..., :3, :3] = r
    T[..., 3, 3] = 1.0
    local = np.einsum('beij,ejk->beik', T,
                      tip_to_base.astype(np.float32)).astype(np.float32)
    worlds = np.zeros((b, n, 4, 4), np.float32)
    worlds[:, 0] = np.eye(4, dtype=np.float32)
    for i in range(1, n):
        par = (i - 1) // 2
        worlds[:, i] = (worlds[:, par] @ local[:, i - 1]).astype(np.float32)
    return worlds


# --------------------------------------------------------------------------- #
# device kernel build
# --------------------------------------------------------------------------- #
def _build_nc(general_constraints: bool, sc_const: float, of_const: float):
    import concourse.bacc as bacc
    import concourse.mybir as mybir
    from concourse.tile import TileContext
    import concourse.bass as bass

    f32 = mybir.dt.float32
    Alu = mybir.AluOpType
    AFT = mybir.ActivationFunctionType

    nc = bacc.Bacc("TRN2", target_bir_lowering=False, debug=False)

    la_d = nc.dram_tensor("la", [BPC, J], f32, kind="ExternalInput")
    tip_d = nc.dram_tensor("tip", [1, E * 12], f32, kind="ExternalInput")
    if general_constraints:
        cs_d = nc.dram_tensor("cs", [1, J], f32, kind="ExternalInput")
        co_d = nc.dram_tensor("co", [1, J], f32, kind="ExternalInput")
    out_d = nc.dram_tensor("out", [BPC, N * 16], f32, kind="ExternalOutput")

    def eng(tag):
        return nc.vector if tag == "v" else nc.gpsimd

    with TileContext(nc) as tc:
        with tc.tile_pool(name="main", bufs=1) as pool, \
             tc.tile_pool(name="scr", bufs=2) as scr:

            la_t = pool.tile([P, NSUB * J], f32)    # input; later wrap output
            th_t = pool.tile([P, NSUB * J], f32)    # tanh; later cos
            s_t = pool.tile([P, NSUB * J], f32)     # sin
            tip_t = pool.tile([P, E * 12], f32)     # broadcast tip rows
            loc_t = [pool.tile([P, E * 12], f32, tag=f"loc{s}", name=f"loc{s}") for s in range(NSUB)]
            w_t = [pool.tile([P, N * 16], f32, tag=f"w{s}", name=f"w{s}") for s in range(NSUB)]
            if general_constraints:
                cs_t = pool.tile([P, J], f32)
                co_t = pool.tile([P, J], f32)

            # ---------------- input DMAs ----------------
            la_v = la_d[:].rearrange("(s p) j -> p s j", p=P)    # [128, 4, 765]
            for s in range(NSUB):
                nc.sync.dma_start(la_t[:, s * J:(s + 1) * J], la_v[:, s])
            tip_src = bass.AP(tip_d, 0, [[0, P], [1, E * 12]])   # bcast partitions
            nc.sync.dma_start(tip_t[:], tip_src)
            if general_constraints:
                nc.sync.dma_start(cs_t[:], bass.AP(cs_d, 0, [[0, P], [1, J]]))
                nc.sync.dma_start(co_t[:], bass.AP(co_d, 0, [[0, P], [1, J]]))

            # ---------------- stage A: theta, sin, cos ----------------
            act = nc.scalar.activation
            act(th_t[:], la_t[:], AFT.Tanh)
            if general_constraints:
                # theta = th*scale + offset, elementwise per angle j
                for s in range(NSUB):
                    blk = th_t[:, s * J:(s + 1) * J]
                    nc.vector.tensor_tensor(blk, blk, cs_t[:], Alu.mult)
                    nc.vector.tensor_tensor(blk, blk, co_t[:], Alu.add)
                scv, ofv = 1.0, 0.0
            else:
                scv, ofv = sc_const, of_const
            # sin(theta) = sin(scv*th + ofv)
            act(s_t[:], th_t[:], AFT.Sin, bias=ofv, scale=scv)
            # cos(theta) = sin(scv*wrap(th + (ofv + pi/2)/scv))
            nc.vector.add_range_wrap(la_t[:], th_t[:],
                                     (ofv + PI / 2.0) / scv, PI / scv,
                                     2.0 * PI / scv)
            act(th_t[:], la_t[:], AFT.Sin, scale=scv)   # cos -> th_t

            # ---------------- stage BC: locals = Rx Ry Rz @ tip ----------------
            tip3 = tip_t[:].rearrange("p (e i l) -> p e i l", e=E, i=3, l=4)
            T0, T1, T2 = (tip3[:, :, i, :] for i in range(3))

            for s in range(NSUB):
                ev = eng(ENG_BC[s])
                j0 = s * J

                def trig(tile, axis):
                    return tile[:, j0 + axis: j0 + J: 3].to_broadcast([P, E, 4])

                sx, sy, sz = (trig(s_t, a) for a in range(3))
                cx, cy, cz = (trig(th_t, a) for a in range(3))

                r0 = scr.tile([P, E * 4], f32, tag="r0", name=f"r0_{s}")
                r1 = scr.tile([P, E * 4], f32, tag="r1", name=f"r1_{s}")
                q2 = scr.tile([P, E * 4], f32, tag="q2", name=f"q2_{s}")
                tA = scr.tile([P, E * 4], f32, tag="tA", name=f"tA_{s}")
                r0v = r0[:].rearrange("p (e l) -> p e l", e=E, l=4)
                r1v = r1[:].rearrange("p (e l) -> p e l", e=E, l=4)
                q2v = q2[:].rearrange("p (e l) -> p e l", e=E, l=4)
                tAv = tA[:].rearrange("p (e l) -> p e l", e=E, l=4)
                loc4 = loc_t[s][:].rearrange("p (e i l) -> p e i l", e=E, i=3, l=4)
                L0, L1, L2 = (loc4[:, :, i, :] for i in range(3))

                tt = ev.tensor_tensor
                # rows of Rz @ tip
                tt(tAv, cz, T0, Alu.mult)
                tt(r0v, sz, T1, Alu.mult)
                tt(r0v, tAv, r0v, Alu.subtract)      # r0 = cz*T0 - sz*T1
                tt(tAv, sz, T0, Alu.mult)
                tt(r1v, cz, T1, Alu.mult)
                tt(r1v, tAv, r1v, Alu.add)           # r1 = sz*T0 + cz*T1
                # rows of Ry @ (Rz tip):  q0 -> locals row0, q2 scratch
                tt(tAv, cy, r0v, Alu.mult)
                tt(L0, sy, T2, Alu.mult)
                tt(L0, L0, tAv, Alu.add)             # q0 = cy*r0 + sy*T2
                tt(tAv, sy, r0v, Alu.mult)
                tt(q2v, cy, T2, Alu.mult)
                tt(q2v, q2v, tAv, Alu.subtract)      # q2 = cy*T2 - sy*r0
                # rows of Rx @ (...):  p1 -> locals row1, p2 -> locals row2
                tt(tAv, cx, r1v, Alu.mult)
                tt(L1, sx, q2v, Alu.mult)
                tt(L1, tAv, L1, Alu.subtract)        # p1 = cx*r1 - sx*q2
                tt(tAv, sx, r1v, Alu.mult)
                tt(L2, cx, q2v, Alu.mult)
                tt(L2, L2, tAv, Alu.add)             # p2 = sx*r1 + cx*q2

            # ---------------- stage D: tree composition ----------------
            for s in range(NSUB):
                ev = eng(ENG_TREE[s])
                tt = ev.tensor_tensor
                wt = w_t[s]
                lt = loc_t[s]
                w4 = wt[:].rearrange("p (n i l) -> p n i l", n=N, i=4, l=4)
                loc4 = lt[:].rearrange("p (e i l) -> p e i l", e=E, i=3, l=4)
                wap = wt[:]
                lap = lt[:]
                wpdim = list(wap.ap[0])
                lpdim = list(lap.ap[0])
                woff = wap.offset
                loff = lap.offset

                def wAP(off, dims):
                    return bass.AP(wap.tensor, woff + off, [list(wpdim)] + dims)

                def lAP(off, dims):
                    return bass.AP(lap.tensor, loff + off, [list(lpdim)] + dims)

                # init: bottom rows (0,0,0,1); root = I
                nc.vector.memset(w4[:, :, 3, 0:3], 0.0)
                nc.vector.memset(w4[:, :, 3, 3], 1.0)
                nc.vector.memset(w4[:, 0, 0:3, :], 0.0)
                nc.vector.memset(wAP(0, [[5, 3]]), 1.0)      # diag of root rot
                # nodes 1,2: W = local
                nc.vector.tensor_copy(w4[:, 1:3, 0:3, :], loc4[:, 0:2, :, :])

                tmp = scr.tile([P, 64 * 12], f32, tag="ttmp", name=f"ttmp_{s}")

                chunks = []
                for (llo, lhi) in [(3, 7), (7, 15), (15, 31), (31, 63),
                                   (63, 127), (127, 255)]:
                    for c0 in range(llo, lhi, 64):
                        chunks.append((c0, min(c0 + 64, lhi)))
                for (lo, hi) in chunks:
                    m = hi - lo
                    q = m // 2
                    plo = (lo - 1) // 2
                    tmpv = tmp[:].rearrange("p (n i l) -> p n i l",
                                            n=64, i=3, l=4)[:, 0:m, :, :]
                    for k in range(3):
                        # Wpar[:, i, k] broadcast over l
                        wp = w4[:, plo:plo + q, 0:3, k].to_broadcast([P, q, 3, 4])
                        for side in (0, 1):
                            # L[child, k, :] broadcast over i (dim inserted)
                            lsrc = lAP((lo - 1 + side) * 12 + k * 4,
                                       [[24, q], [0, 3], [1, 4]])
                            if k == 0:
                                dst = wAP((lo + side) * 16,
                                          [[32, q], [4, 3], [1, 4]])
                            else:
                                tap = tmp[:]
                                dst = bass.AP(tap.tensor,
                                              tap.offset + side * 12,
                                              [list(tap.ap[0]),
                                               [24, q], [4, 3], [1, 4]])
                            tt(dst, wp, lsrc, Alu.mult)
                        if k > 0:
                            wdst = w4[:, lo:hi, 0:3, :]
                            tt(wdst, wdst, tmpv, Alu.add)
                    # translation column: += Wpar[:, i, 3]
                    wtr = wAP(lo * 16 + 3, [[32, q], [16, 2], [4, 3]])
                    ptr = wAP(plo * 16 + 3, [[16, q], [0, 2], [4, 3]])
                    tt(wtr, wtr, ptr, Alu.add)

                # node 255 (single left child of 127)
                wp255 = w4[:, 127, 0:3, :]          # [P,3,4]
                for k in range(3):
                    wpk = w4[:, 127, 0:3, k].to_broadcast([P, 3, 4])
                    lsrc = lAP(254 * 12 + k * 4, [[0, 3], [1, 4]])
                    if k == 0:
                        tt(w4[:, 255, 0:3, :], wpk, lsrc, Alu.mult)
                    else:
                        t255 = tmp[:].rearrange("p (n i l) -> p n i l",
                                                n=64, i=3, l=4)[:, 0, :, :]
                        tt(t255, wpk, lsrc, Alu.mult)
                        tt(w4[:, 255, 0:3, :], w4[:, 255, 0:3, :], t255,
                           Alu.add)
                tt(wAP(255 * 16 + 3, [[4, 3]]),
                   wAP(255 * 16 + 3, [[4, 3]]),
                   wAP(127 * 16 + 3, [[4, 3]]), Alu.add)

            # ---------------- output DMAs ----------------
            out_v = out_d[:].rearrange("(s p) m -> p s m", p=P)  # [128,4,4096]
            for s in range(NSUB):
                for h in range(2):
                    nc.sync.dma_start(
                        out_v[:, s, h * 2048:(h + 1) * 2048],
                        w_t[s][:, h * 2048:(h + 1) * 2048])

    nc.compile()
    return nc


# --------------------------------------------------------------------------- #
# cached PJRT runner (axon path) — compile once, execute per call
# --------------------------------------------------------------------------- #
def _get_runner(general_constraints, sc_const, of_const):
    key = ("runner", general_constraints, round(sc_const, 6), round(of_const, 6))
    if key in _state:
        return _state[key]

    import jax
    from jax.sharding import Mesh, PartitionSpec, NamedSharding
    from jax.experimental.shard_map import shard_map
    import concourse.mybir as mybir
    from concourse import bass2jax

    nc = _build_nc(general_constraints, sc_const, of_const)
    bass2jax.install_neuronx_cc_hook()

    part_name = (nc.partition_id_tensor.name
                 if nc.partition_id_tensor is not None else None)
    in_names, out_names, out_avals = [], [], []
    for alloc in nc.m.functions[0].allocations:
        if not isinstance(alloc, mybir.MemoryLocationSet):
            continue
        name = alloc.memorylocations[0].name
        if alloc.kind == "ExternalInput":
            if name != part_name:
                in_names.append(name)
        elif alloc.kind == "ExternalOutput":
            out_names.append(name)
            out_avals.append(jax.core.ShapedArray(
                tuple(alloc.tensor_shape), mybir.dt.np(alloc.dtype)))
    n_params = len(in_names)
    all_in = in_names + out_names
    if part_name is not None:
        all_in = all_in + [part_name]

    def _body(*args):
        operands = list(args)
        if part_name is not None:
            operands.append(bass2jax.partition_id_tensor())
        outs = bass2jax._bass_exec_p.bind(
            *operands,
            out_avals=tuple(out_avals),
            in_names=tuple(all_in),
            out_names=tuple(out_names),
            lowering_input_output_aliases=(),
            sim_require_finite=True,
            sim_require_nnan=True,
            nc=nc,
        )
        return tuple(outs)

    devices = jax.devices()[:NCORE]
    mesh = Mesh(np.asarray(devices), ("core",))
    nin = n_params + len(out_names)
    sharded = jax.jit(
        shard_map(_body, mesh=mesh,
                  in_specs=(PartitionSpec("core"),) * nin,
                  out_specs=(PartitionSpec("core"),) * len(out_names),
                  check_rep=False),
        donate_argnums=tuple(range(n_params, nin)),
        keep_unused=True,
    )
    shard0 = NamedSharding(mesh, PartitionSpec("core"))

    def _make_zeros():
        return jax.jit(
            lambda: jax.numpy.zeros((NCORE * BPC, N * 16), np.float32),
            out_shardings=shard0)()

    runner = (sharded, in_names, _make_zeros)
    _state[key] = runner
    return runner


def _run_device(log_angles, tip_rows, cs, co, general_constraints,
                sc_const, of_const):
    sharded, in_names, make_zeros = _get_runner(
        general_constraints, sc_const, of_const)
    feed = {
        "la": np.ascontiguousarray(log_angles, dtype=np.float32),
        "tip": np.broadcast_to(tip_rows.reshape(1, E * 12),
                               (NCORE, E * 12)).copy(),
    }
    if general_constraints:
        feed["cs"] = np.broadcast_to(cs.reshape(1, J), (NCORE, J)).copy()
        feed["co"] = np.broadcast_to(co.reshape(1, J), (NCORE, J)).copy()
    args = [feed[name] for name in in_names]
    out = sharded(*args, make_zeros())[0]
    return np.asarray(out).reshape(B, N, 4, 4)


# --------------------------------------------------------------------------- #
# public entry point
# --------------------------------------------------------------------------- #
def kernel(log_angles, tip_to_base, rot_axes, rot_constraints):
    log_angles = np.asarray(log_angles)
    tip_to_base = np.asarray(tip_to_base)
    rot_axes = np.asarray(rot_axes)
    rot_constraints = np.asarray(rot_constraints)

    expected_shapes = (log_angles.shape == (B, J)
                       and tip_to_base.shape == (E, 4, 4)
                       and rot_axes.shape == (J, 3)
                       and rot_constraints.shape == (J, 2))
    eye_tiled = np.tile(np.eye(3, dtype=np.float32), (E, 1)) \
        if expected_shapes else None
    euler = expected_shapes and np.allclose(rot_axes, eye_tiled, atol=1e-6)
    if not euler:
        return _np_fallback(log_angles, tip_to_base, rot_axes, rot_constraints)

    sc = rot_constraints[:, 0].astype(np.float32)
    of = rot_constraints[:, 1].astype(np.float32)
    const_ok = (np.all(sc == sc[0]) and np.all(of == of[0])
                and sc[0] > 1e-3 and abs(sc[0]) + abs(of[0]) <= PI + 1e-4)
    general = not const_ok

    tip_rows = np.ascontiguousarray(
        tip_to_base[:, :3, :], dtype=np.float32)          # (E, 3, 4)

    out = _run_device(log_angles, tip_rows,
                      sc if general else None, of if general else None,
                      general,
                      float(sc[0]) if const_ok else 1.0,
                      float(of[0]) if const_ok else 0.0)
    return out


# revision 32
# speedup vs baseline: 7991.6936x; 7991.6936x over previous
"""Trainium2 Bass kernel for nn_BodyKinematics (batched tree forward kinematics).

Contract: kernel(**inputs) takes the FULL unsharded inputs as numpy arrays and
returns the FULL output (B, N, 4, 4) float32.  Internally the batch dim is
sharded across 8 NeuronCores (pure data parallelism); the tiny per-edge
parameters are replicated.

Math (matches the jax reference):
  theta = tanh(log_angles) * scale + offset            # (B, 3E)
  per edge e: r = Rx(th_x) @ Ry(th_y) @ Rz(th_z)       # axes are e_x, e_y, e_z
  local_e  = [r | 0; 0 1] @ tip_to_base[e]             # affine 3x4 is enough
  tree: W_0 = I, W_n = W_parent(n) @ local_{n-1}       # parent(n) = (n-1)//2
  output: W as 4x4 with constant bottom row (0,0,0,1)

Device layout (per core, 512 batch rows = 4 subtiles x 128 partitions):
  partitions = batch-within-subtile, free dim = per-edge structure.
  locals tile (per sub):  [128, E*12]   (e, i, l) row-major
  worlds tile (per sub):  [128, N*16]   (n, i, l) -> 16KB contiguous per batch
                                         row for efficient output DMA
"""

import os
import sys

for _p in ("/opt/trn_rl_repo",):
    if _p not in sys.path and os.path.isdir(_p):
        sys.path.insert(0, _p)

import numpy as np

B, E, N = 4096, 255, 256
J = 3 * E           # 765 angles
NCORE, P, NSUB = 8, 128, 4
BPC = P * NSUB      # 512 batch rows per core
PI = float(np.pi)

# engine assignment: "v" = VectorE (DVE), "g" = GpSimd (POOL).
# BC_ENG: per op-triple (tA_mul, indep_mul, combine); same for all subs.
BC_ENG = [("v", "v", "v")] * 6
# tree per-level op classes: k=0 muls L/R, k=1 muls L/R + add, k=2 muls L/R
# + add, translation add.
TREE_ENG = {"m0L": "v", "m0R": "v", "t1L": "v", "t1R": "v", "a1": "v",
            "t2L": "v", "t2R": "v", "a2": "v", "tr": "v"}
MEMSET_ENG = "v"
TREE_SUB_ENG = ["g", "v", "v", "v"]
# per-sub, per-level engine override (7 levels + tail); None -> use
# TREE_SUB_ENG for every level
TREE_LVL_ENG = {}
BC_ALT = False    # alternate engine roles between BC groups
TREE_ALT = False  # alternate engine roles between tree groups
GS = 1      # subs per BC op-group
TGS = 2     # subs per tree op-group
COS_MODE = "abs"
STAGE_LP = False  # ACT-stage locals into PSUM for DVE tree levels          # "abs" (1 TS op @2x) or "wrap" (custom DVE op @1x)
# debug: skip stages to attribute time in TimelineSim
SKIP = set()              # subset of {"A", "BC", "TREE", "OUT"}

_state: dict = {}


# --------------------------------------------------------------------------- #
# numpy fallback (exact float32 port of the reference) — used only if the
# inputs don't match the structure the device kernel was built for.
# --------------------------------------------------------------------------- #
def _np_skew(a):
    x, y, z = a[..., 0], a[..., 1], a[..., 2]
    zero = np.zeros_like(x)
    return np.stack([
        np.stack([zero, -z, y], -1),
        np.stack([z, zero, -x], -1),
        np.stack([-y, x, zero], -1)], -2)


def _np_fallback(log_angles, tip_to_base, rot_axes, rot_constraints):
    la = log_angles.astype(np.float32)
    b, e3 = la.shape
    e = e3 // 3
    n = e + 1
    theta = np.tanh(la) * rot_constraints[:, 0] + rot_constraints[:, 1]
    K = _np_skew(rot_axes.astype(np.float32))
    K2 = np.einsum('mij,mjk->mik', K, K).astype(np.float32)
    s = np.sin(theta)[..., None, None]
    c = (1.0 - np.cos(theta))[..., None, None]
    I3 = np.eye(3, dtype=np.float32)
    rots = (I3 + s * K + c * K2).reshape(b, e, 3, 3, 3).astype(np.float32)
    r = np.einsum('beij,bejk,bekl->beil', rots[:, :, 0], rots[:, :, 1],
                  rots[:, :, 2]).astype(np.float32)
    T = np.zeros((b, e, 4, 4), np.float32)
    T[..., :3, :3] = r
    T[..., 3, 3] = 1.0
    local = np.einsum('beij,ejk->beik', T,
                      tip_to_base.astype(np.float32)).astype(np.float32)
    worlds = np.zeros((b, n, 4, 4), np.float32)
    worlds[:, 0] = np.eye(4, dtype=np.float32)
    for i in range(1, n):
        par = (i - 1) // 2
        worlds[:, i] = (worlds[:, par] @ local[:, i - 1]).astype(np.float32)
    return worlds


# --------------------------------------------------------------------------- #
# device kernel build
# --------------------------------------------------------------------------- #
def _build_nc(general_constraints: bool, sc_const: float, of_const: float,
              loop_n: int = 1):
    import concourse.bacc as bacc
    import concourse.mybir as mybir
    from concourse.tile import TileContext
    import concourse.bass as bass
    from contextlib import ExitStack

    f32 = mybir.dt.float32
    i32 = mybir.dt.int32
    Alu = mybir.AluOpType
    AFT = mybir.ActivationFunctionType

    nc = bacc.Bacc("TRN2", target_bir_lowering=False, debug=False)

    la_d = nc.dram_tensor("la", [BPC, J], f32, kind="ExternalInput")
    tip_d = nc.dram_tensor("tip", [1, E * 12], f32, kind="ExternalInput")
    if general_constraints:
        cs_d = nc.dram_tensor("cs", [1, J], f32, kind="ExternalInput")
        co_d = nc.dram_tensor("co", [1, J], f32, kind="ExternalInput")
    out_d = nc.dram_tensor("out", [BPC, N * 16], f32, kind="ExternalOutput")

    def eng(tag):
        return nc.vector if tag == "v" else nc.gpsimd

    with TileContext(nc) as tc:
        with tc.tile_pool(name="main", bufs=1) as pool, \
             tc.tile_pool(name="scr", bufs=2) as scr, \
             ExitStack() as _loop_ctx:
            if loop_n > 1:
                _loop_ctx.enter_context(tc.For_i(0, loop_n, 1))

            la_t = pool.tile([P, NSUB * J], f32)    # input; reused as |t|
            th_t = pool.tile([P, NSUB * J], f32)    # tanh
            tip_t = pool.tile([P, E * 12], f32)     # broadcast tip rows
            loc_t = [pool.tile([P, E * 12], f32, tag=f"loc{s}",
                               name=f"loc{s}") for s in range(NSUB)]
            w_t = [pool.tile([P, N * 16], f32, tag=f"w{s}", name=f"w{s}")
                   for s in range(NSUB)]
            if general_constraints:
                cs_t = pool.tile([P, J], f32)
                co_t = pool.tile([P, J], f32)

            # ---------------- input DMAs ----------------
            la_v = la_d[:].rearrange("(s p) j -> p s j", p=P)    # [128, 4, 765]
            for s in range(NSUB):
                nc.sync.dma_start(la_t[:, s * J:(s + 1) * J], la_v[:, s])
            tip_src = bass.AP(tip_d, 0, [[0, P], [1, E * 12]])
            nc.sync.dma_start(tip_t[:], tip_src)
            if general_constraints:
                nc.sync.dma_start(cs_t[:], bass.AP(cs_d, 0, [[0, P], [1, J]]))
                nc.sync.dma_start(co_t[:], bass.AP(co_d, 0, [[0, P], [1, J]]))

            # ---------------- stage A: theta -> tanh ----------------
            act = nc.scalar.activation
            act(th_t[:], la_t[:], AFT.Tanh)
            if general_constraints:
                for s in range(NSUB):
                    blk = th_t[:, s * J:(s + 1) * J]
                    nc.vector.tensor_tensor(blk, blk, cs_t[:], Alu.mult)
                    nc.vector.tensor_tensor(blk, blk, co_t[:], Alu.add)
                scv, ofv = 1.0, 0.0
            else:
                scv, ofv = sc_const, of_const
            if ofv == 0.0:
                ofv_ap = None
            else:
                ofv_t = pool.tile([P, 1], f32)
                nc.gpsimd.memset(ofv_t[:], ofv)
                ofv_ap = ofv_t[:]
            use_abs = (COS_MODE == "abs" and ofv == 0.0)
            if use_abs:
                hpi_t = pool.tile([P, 1], f32)
                nc.gpsimd.memset(hpi_t[:], PI / 2.0)
                # |t| for the whole tile in one 2x op (cos input)
                nc.vector.tensor_scalar(
                    la_t[:].bitcast(i32), th_t[:].bitcast(i32),
                    0x7FFFFFFF, None, Alu.bitwise_and)
            else:
                nc.vector.add_range_wrap(la_t[:], th_t[:],
                                         (ofv + PI / 2.0) / scv, PI / scv,
                                         2.0 * PI / scv)

            # ---------------- BC + tree, pipelined per sub ----------------
            tip3 = tip_t[:].rearrange("p (e i l) -> p e i l", e=E, i=3, l=4)
            T0, T1, T2 = (tip3[:, :, i, :] for i in range(3))
            _bc_ps = tc.tile_pool(name="bcps", bufs=2, space="PSUM")
            psp2 = _bc_ps.__enter__()
            _bc_ps1 = tc.tile_pool(name="bcps1", bufs=1, space="PSUM")
            psp = _bc_ps1.__enter__()
            tA = psp.tile([P, E * 4], f32)           # shared PSUM scratch
            tAv = tA[:].rearrange("p (e l) -> p e l", e=E, l=4)

            for s in range(NSUB):
                j0 = s * J
                # per-sub sin/cos in PSUM, written by ACT (own ports)
                sc_ps = psp2.tile([P, 2 * J], f32, tag="scps",
                                  name=f"scps_{s}")
                sin_ap = sc_ps[:, 0:J]
                cos_ap = sc_ps[:, J:2 * J]
                if ofv_ap is None:
                    act(sin_ap, th_t[:, j0:j0 + J], AFT.Sin, scale=scv)
                else:
                    act(sin_ap, th_t[:, j0:j0 + J], AFT.Sin, bias=ofv_ap,
                        scale=scv)
                if use_abs:
                    act(cos_ap, la_t[:, j0:j0 + J], AFT.Sin, bias=hpi_t[:],
                        scale=-scv)
                else:
                    act(cos_ap, la_t[:, j0:j0 + J], AFT.Sin, scale=scv)

                def trig(base, axis):
                    return base[:, axis::3].to_broadcast([P, E, 4])

                sx, sy, sz = (trig(sin_ap, a) for a in range(3))
                cx, cy, cz = (trig(cos_ap, a) for a in range(3))

                r0 = scr.tile([P, E * 4], f32, tag="r0", name=f"r0_{s}")
                r1 = scr.tile([P, E * 4], f32, tag="r1", name=f"r1_{s}")
                q2 = scr.tile([P, E * 4], f32, tag="q2", name=f"q2_{s}")
                r0v, r1v, q2v = (
                    t[:].rearrange("p (e l) -> p e l", e=E, l=4)
                    for t in (r0, r1, q2))
                loc4 = loc_t[s][:].rearrange("p (e i l) -> p e i l",
                                             e=E, i=3, l=4)
                L0, L1, L2 = (loc4[:, :, i, :] for i in range(3))

                # all on DVE; every op reads <=1 SBUF stream (trig + tA are
                # PSUM), so POOL can run other subs' trees concurrently.
                tt = nc.vector.tensor_tensor
                triples = [
                    (cz, T0, sz, T1, r0v, Alu.subtract, True),
                    (sz, T0, cz, T1, r1v, Alu.add, False),
                    (cy, r0v, sy, T2, L0, Alu.add, False),
                    (sy, r0v, cy, T2, q2v, Alu.subtract, False),
                    (cx, r1v, sx, q2v, L1, Alu.subtract, True),
                    (sx, r1v, cx, q2v, L2, Alu.add, False),
                ]
                for (a, b, c, d, dst, op, ta_first) in triples:
                    tt(tAv, a, b, Alu.mult)
                    tt(dst, c, d, Alu.mult)
                    if ta_first:
                        tt(dst, tAv, dst, op)
                    else:
                        tt(dst, dst, tAv, op)

            _bc_ps1.__exit__(None, None, None)
            _bc_ps.__exit__(None, None, None)
            _tr_ps = tc.tile_pool(name="trps", bufs=2, space="PSUM")
            pst = _tr_ps.__enter__()

            # ---------------- tree, per sub; engine per TREE_SUB_ENG -------
            for s in range(NSUB):
                etag = TREE_SUB_ENG[s]
                lvl_tags = TREE_LVL_ENG.get(
                    s, [etag] * 8)
                ev = eng(etag)
                wt = w_t[s]
                lt = loc_t[s]
                w4 = wt[:].rearrange("p (n i l) -> p n i l", n=N, i=4, l=4)
                loc4 = lt[:].rearrange("p (e i l) -> p e i l", e=E, i=3, l=4)
                wap = wt[:]
                lap = lt[:]
                wpdim = list(wap.ap[0])
                lpdim = list(lap.ap[0])
                woff = wap.offset
                loff = lap.offset

                def wAP(off, dims):
                    return bass.AP(wap.tensor, woff + off,
                                   [list(wpdim)] + dims)

                def lAP(off, dims):
                    return bass.AP(lap.tensor, loff + off,
                                   [list(lpdim)] + dims)

                ev.memset(w4[:, :, 3, 0:3], 0.0)
                ev.memset(w4[:, :, 3, 3], 1.0)
                ev.memset(w4[:, 0, 0:3, :], 0.0)
                ev.memset(wAP(0, [[5, 3]]), 1.0)      # root rot diag
                ev.tensor_copy(w4[:, 1:3, 0:3, :], loc4[:, 0:2, :, :])

                tmps = {}
                for tg in set(lvl_tags):
                    if tg == "v" and STAGE_LP:
                        tmps["v"] = pst.tile([P, 64 * 12], f32,
                                             tag="ttmp_ps",
                                             name=f"ttmpv_{s}")
                    else:
                        tmps[tg] = scr.tile([P, 64 * 12], f32, tag="ttmp",
                                            name=f"ttmpg_{s}")

                for li, (lo, hi) in enumerate(
                        [(3, 7), (7, 15), (15, 31), (31, 63),
                         (63, 127), (127, 191), (191, 255)]):
                    ltag = lvl_tags[li]
                    tt = eng(ltag).tensor_tensor
                    tmp = tmps[ltag]
                    use_psum = (ltag == "v") and STAGE_LP
                    m = hi - lo
                    q = m // 2
                    plo = (lo - 1) // 2
                    if use_psum:
                        # stage this level's locals into PSUM via ACT so the
                        # DVE muls read only one SBUF stream
                        lp = pst.tile([P, 64 * 12], f32, tag="lp_ps",
                                      name=f"lp_{s}_{lo}")
                        nc.scalar.copy(lp[:, 0:m * 12],
                                       lt[:, (lo - 1) * 12:(hi - 1) * 12])
                        lsrc_base = lp[:]
                        lsoff = lp[:].offset
                        lspd = list(lp[:].ap[0])

                        def lsAP(off, dims):
                            return bass.AP(lsrc_base.tensor, lsoff + off,
                                           [list(lspd)] + dims)
                    tmpv = tmp[:].rearrange("p (n i l) -> p n i l",
                                            n=64, i=3, l=4)[:, 0:m, :, :]
                    for k in range(3):
                        wp = w4[:, plo:plo + q, 0:3, k].to_broadcast(
                            [P, q, 3, 4])
                        for side in (0, 1):
                            if use_psum:
                                lsrc = lsAP(side * 12 + k * 4,
                                            [[24, q], [0, 3], [1, 4]])
                            else:
                                lsrc = lAP((lo - 1 + side) * 12 + k * 4,
                                           [[24, q], [0, 3], [1, 4]])
                            if k == 0:
                                dst = wAP((lo + side) * 16,
                                          [[32, q], [4, 3], [1, 4]])
                            else:
                                tap = tmp[:]
                                dst = bass.AP(tap.tensor,
                                              tap.offset + side * 12,
                                              [list(tap.ap[0]),
                                               [24, q], [4, 3], [1, 4]])
                            tt(dst, wp, lsrc, Alu.mult)
                        if k > 0:
                            wdst = w4[:, lo:hi, 0:3, :]
                            tt(wdst, wdst, tmpv, Alu.add)
                    wtr = wAP(lo * 16 + 3, [[32, q], [16, 2], [4, 3]])
                    ptr = wAP(plo * 16 + 3, [[16, q], [0, 2], [4, 3]])
                    tt(wtr, wtr, ptr, Alu.add)

                # node 255 (single left child of 127)
                tt = eng(lvl_tags[7]).tensor_tensor
                tmp = tmps[lvl_tags[7]]
                for k in range(3):
                    wpk = w4[:, 127, 0:3, k].to_broadcast([P, 3, 4])
                    lsrc = lAP(254 * 12 + k * 4, [[0, 3], [1, 4]])
                    if k == 0:
                        tt(w4[:, 255, 0:3, :], wpk, lsrc, Alu.mult)
                    else:
                        t255 = tmp[:].rearrange("p (n i l) -> p n i l",
                                                n=64, i=3, l=4)[:, 0, :, :]
                        tt(t255, wpk, lsrc, Alu.mult)
                        tt(w4[:, 255, 0:3, :], w4[:, 255, 0:3, :], t255,
                           Alu.add)
                tt(wAP(255 * 16 + 3, [[4, 3]]),
                   wAP(255 * 16 + 3, [[4, 3]]),
                   wAP(127 * 16 + 3, [[4, 3]]), Alu.add)

            _tr_ps.__exit__(None, None, None)

            # ---------------- output DMAs ----------------
            out_v = out_d[:].rearrange("(s p) m -> p s m", p=P)  # [128,4,4096]
            for s in range(NSUB):
                for h in range(2):
                    nc.sync.dma_start(
                        out_v[:, s, h * 2048:(h + 1) * 2048],
                        w_t[s][:, h * 2048:(h + 1) * 2048])

    nc.compile()
    return nc


# --------------------------------------------------------------------------- #
# cached PJRT runner (axon path) — compile once, execute per call
# --------------------------------------------------------------------------- #
def _get_runner(general_constraints, sc_const, of_const, loop_n=1):
    key = ("runner", general_constraints, round(sc_const, 6), round(of_const, 6), loop_n)
    if key in _state:
        return _state[key]

    import jax
    from jax.sharding import Mesh, PartitionSpec, NamedSharding
    from jax.experimental.shard_map import shard_map
    import concourse.mybir as mybir
    from concourse import bass2jax

    nc = _build_nc(general_constraints, sc_const, of_const, loop_n)
    bass2jax.install_neuronx_cc_hook()

    part_name = (nc.partition_id_tensor.name
                 if nc.partition_id_tensor is not None else None)
    in_names, out_names, out_avals = [], [], []
    for alloc in nc.m.functions[0].allocations:
        if not isinstance(alloc, mybir.MemoryLocationSet):
            continue
        name = alloc.memorylocations[0].name
        if alloc.kind == "ExternalInput":
            if name != part_name:
                in_names.append(name)
        elif alloc.kind == "ExternalOutput":
            out_names.append(name)
            out_avals.append(jax.core.ShapedArray(
                tuple(alloc.tensor_shape), mybir.dt.np(alloc.dtype)))
    n_params = len(in_names)
    all_in = in_names + out_names
    if part_name is not None:
        all_in = all_in + [part_name]

    def _body(*args):
        operands = list(args)
        if part_name is not None:
            operands.append(bass2jax.partition_id_tensor())
        outs = bass2jax._bass_exec_p.bind(
            *operands,
            out_avals=tuple(out_avals),
            in_names=tuple(all_in),
            out_names=tuple(out_names),
            lowering_input_output_aliases=(),
            sim_require_finite=True,
            sim_require_nnan=True,
            nc=nc,
        )
        return tuple(outs)

    devices = jax.devices()[:NCORE]
    mesh = Mesh(np.asarray(devices), ("core",))
    nin = n_params + len(out_names)
    sharded = jax.jit(
        shard_map(_body, mesh=mesh,
                  in_specs=(PartitionSpec("core"),) * nin,
                  out_specs=(PartitionSpec("core"),) * len(out_names),
                  check_rep=False),
        donate_argnums=tuple(range(n_params, nin)),
        keep_unused=True,
    )
    shard0 = NamedSharding(mesh, PartitionSpec("core"))

    def _make_zeros():
        return jax.jit(
            lambda: jax.numpy.zeros((NCORE * BPC, N * 16), np.float32),
            out_shardings=shard0)()

    runner = (sharded, in_names, _make_zeros)
    _state[key] = runner
    return runner


def _run_device(log_angles, tip_rows, cs, co, general_constraints,
                sc_const, of_const, loop_n=1):
    sharded, in_names, make_zeros = _get_runner(
        general_constraints, sc_const, of_const, loop_n)
    feed = {
        "la": np.ascontiguousarray(log_angles, dtype=np.float32),
        "tip": np.broadcast_to(tip_rows.reshape(1, E * 12),
                               (NCORE, E * 12)).copy(),
    }
    if general_constraints:
        feed["cs"] = np.broadcast_to(cs.reshape(1, J), (NCORE, J)).copy()
        feed["co"] = np.broadcast_to(co.reshape(1, J), (NCORE, J)).copy()
    args = [feed[name] for name in in_names]
    out = sharded(*args, make_zeros())[0]
    return np.asarray(out).reshape(B, N, 4, 4)


def _bench_device(log_angles, tip_rows, sc_const, of_const, loop_n, reps):
    """Device-only timing: inputs stay on device, outputs never fetched."""
    import time
    import jax

    sharded, in_names, make_zeros = _get_runner(False, sc_const, of_const,
                                                loop_n)
    feed = {
        "la": np.ascontiguousarray(log_angles, dtype=np.float32),
        "tip": np.broadcast_to(tip_rows.reshape(1, E * 12),
                               (NCORE, E * 12)).copy(),
    }
    args = [jax.device_put(feed[n]) for n in in_names]
    # warmup (compile + first exec)
    jax.block_until_ready(sharded(*args, make_zeros()))
    ts = []
    for _ in range(reps):
        z = make_zeros()
        jax.block_until_ready(z)
        t0 = time.time()
        jax.block_until_ready(sharded(*args, z))
        ts.append(time.time() - t0)
    return min(ts)


# --------------------------------------------------------------------------- #
# public entry point
# --------------------------------------------------------------------------- #
def kernel(log_angles, tip_to_base, rot_axes, rot_constraints):
    log_angles = np.asarray(log_angles)
    tip_to_base = np.asarray(tip_to_base)
    rot_axes = np.asarray(rot_axes)
    rot_constraints = np.asarray(rot_constraints)

    expected_shapes = (log_angles.shape == (B, J)
                       and tip_to_base.shape == (E, 4, 4)
                       and rot_axes.shape == (J, 3)
                       and rot_constraints.shape == (J, 2))
    eye_tiled = np.tile(np.eye(3, dtype=np.float32), (E, 1)) \
        if expected_shapes else None
    euler = expected_shapes and np.allclose(rot_axes, eye_tiled, atol=1e-6)
    if not euler:
        return _np_fallback(log_angles, tip_to_base, rot_axes, rot_constraints)

    sc = rot_constraints[:, 0].astype(np.float32)
    of = rot_constraints[:, 1].astype(np.float32)
    const_ok = (np.all(sc == sc[0]) and np.all(of == of[0])
                and float(sc[0]) > 1e-3
                and abs(float(sc[0])) + abs(float(of[0])) <= PI + 1e-4)
    if not const_ok:
        # untested-on-device parameter regime: use the exact host fallback
        return _np_fallback(log_angles, tip_to_base, rot_axes,
                            rot_constraints)

    tip_rows = np.ascontiguousarray(
        tip_to_base[:, :3, :], dtype=np.float32)          # (E, 3, 4)

    out = _run_device(log_angles, tip_rows, None, None, False,
                      float(sc[0]), float(of[0]))
    return out
